# revision 1
# baseline (speedup 1.0000x reference)
"""MDLSTM (4-direction 2D-LSTM) Trainium2 kernel, v2.

Sharding: 8 cores = 4 scan directions x 2 batch halves (B_local=16).
Anti-diagonal wavefronts; cells (i, j) with i+j = t are independent and
depend only on wavefront t-1.

v2 changes vs the original baseline (TimelineSim 2.59 ms -> 0.89 ms):
  - fp16 compute end to end (PE 1 cyc/row at any width, DVE 2x/4x perf
    modes; ~1.1e-2 final rel err vs the 2e-2 budget).
  - x fully resident in SBUF (fp16), loaded once via 8 parallel DMAs.
  - PSUM plane layout [branch][gate i,f,o,g]; per gate the two branches'
    accumulation groups sit in different banks, so they pipeline and the
    stationary weight is shared by consecutive matmuls.
  - ALL gate activations in a single sigmoid instruction per chunk-step:
    gate-g weights are pre-doubled on host and tanh(x) = 2*sigmoid(2x)-1;
    the affine fix runs on the DVE in 4x mode. Only tanh(cn) remains as a
    second ScalE instruction.
  - state double-buffered [buf][c|h][33 slots][16B]: output DMA and next
    step's matmul reads never block state writes; ct and ht are committed
    by one tensor_scalar + one scalar_tensor_tensor.
  - wavefronts split into ~8-row column chunks (4 lanes) so consecutive
    wavefronts pipeline across PE -> ScalE -> DVE; chunk count adapts in
    the ramp wavefronts.
  - h streamed out with one DMA per wavefront (contiguous per-partition
    descriptors).
"""

import numpy as np

B_FULL, CIN, H, W = 32, 16, 32, 128
O = 128
B = 16  # batch per core
N_CORES = 8
NG = 4  # gates i, f, o, g
CHUNKS = 4


def _wavefronts(h, w):
    out = []
    off = 0
    for t in range(h + w - 1):
        i0 = max(0, t - (w - 1))
        i1 = min(h, t + 1)
        out.append((t, i0, i1, off))
        off += (i1 - i0) * B
    return out


def build_module(h, w, chunks=CHUNKS):
    import concourse.bacc as bacc
    import concourse.mybir as mybir
    import concourse.tile as tile

    dt = mybir.dt
    f16 = dt.float16
    f32 = dt.float32
    AF = mybir.ActivationFunctionType
    ALU = mybir.AluOpType

    wfs = _wavefronts(h, w)
    ncols = h * w * B
    nslots = h + 1
    # per-chunk max columns; padded so the two branch plane-groups of a
    # gate never share a PSUM bank (their accumulation groups are open
    # concurrently)
    ckmax = max(-(-h // chunks) * B, 128)

    nc = bacc.Bacc("TRN2", target_bir_lowering=False, debug=False)

    x_diag = nc.dram_tensor("x_diag", [CIN + 1, ncols], f16, kind="ExternalInput")
    whT = nc.dram_tensor("whT", [O, NG * O], f16, kind="ExternalInput")
    wxT = nc.dram_tensor("wxT", [CIN + 1, NG * O], f16, kind="ExternalInput")
    ws0v = nc.dram_tensor("ws0v", [O, 1], f32, kind="ExternalInput")
    ws1v = nc.dram_tensor("ws1v", [O, 1], f32, kind="ExternalInput")
    biasv = nc.dram_tensor("biasv", [O, 1], f32, kind="ExternalInput")
    zerov = nc.dram_tensor("zerov", [O, 2, 2, nslots, B], f16, kind="ExternalInput")
    h_diag = nc.dram_tensor("h_diag", [O, ncols], f16, kind="ExternalOutput")

    with tile.TileContext(nc) as tc:
        with (
            tc.tile_pool(name="const", bufs=1) as cpool,
            tc.tile_pool(name="state", bufs=1) as spool,
            tc.tile_pool(name="gates", bufs=2) as gpool,
            tc.tile_pool(name="work", bufs=2) as wpool,
            tc.tile_pool(name="psum", bufs=1, space="PSUM") as ppool,
        ):
            whT_s = cpool.tile([O, NG * O], f16, tag="whT")
            wxT_s = cpool.tile([CIN + 1, NG * O], f16, tag="wxT")
            ws0_s = cpool.tile([O, 1], f32, tag="ws0")
            ws1_s = cpool.tile([O, 1], f32, tag="ws1")
            bias_s = cpool.tile([O, 1], f32, tag="bias")
            nc.sync.dma_start(whT_s[:], whT[:])
            nc.sync.dma_start(wxT_s[:], wxT[:])
            nc.sync.dma_start(ws0_s[:], ws0v[:])
            nc.sync.dma_start(ws1_s[:], ws1v[:])
            nc.sync.dma_start(bias_s[:], biasv[:])

            # state: [buf][c|h][slot][b]; slot 0 stays zero forever
            sc = spool.tile([O, 2, 2, nslots, B], f16, tag="sc")
            nc.sync.dma_start(sc[:], zerov[:])

            # whole x resident in SBUF; chunked DMAs so early columns land first
            xs = cpool.tile([CIN + 1, ncols], f16, tag="xs")
            nxc = 8
            xstep = -(-ncols // nxc)
            for c in range(nxc):
                lo = c * xstep
                hi = min(ncols, lo + xstep)
                nc.sync.dma_start(xs[:, lo:hi], x_diag[:, lo:hi])

            # gate order [i, f, o, g]; psum/gate plane = 2*gi + branch
            GI, GF, GO, GG = 0, 1, 2, 3

            for t, i0, i1, off in wfs:
                d = i1 - i0
                bp = (t + 1) % 2  # prev state buffer
                bc = t % 2
                # adaptive chunk count: balanced chunks of <= rows_per_chunk
                # rows, so ramp wavefronts don't pay per-instruction
                # overhead `chunks` times
                rows_per_chunk = -(-h // chunks)
                nck = -(-d // rows_per_chunk)
                csz = -(-d // nck)
                db = d * B
                # per-wavefront cn/pp staging and tau: chunks write disjoint
                # column ranges of one tile, so a single tanh instruction
                # covers the whole wavefront (ScalE per-instruction overhead
                # paid once, not per chunk)
                CP = wpool.tile([O, 4, h * B], f16, tag="cp")
                TAU = wpool.tile([O, 2, h * B], f16, tag="tau")
                for k in range(nck):
                    r0 = i0 + k * csz
                    r1 = min(i1, r0 + csz)
                    dk = r1 - r0
                    if dk <= 0:
                        continue
                    ck = dk * B
                    offk = off + (r0 - i0) * B

                    # plane layout [branch][gate i,f,o,g]; the two branch
                    # groups live in different PSUM banks, so per gate the two
                    # branches' accumulation groups can be open concurrently
                    # (and the stationary weight is loaded once per pair).
                    P = ppool.tile([O, 2, 4, ckmax], f32, tag=f"p{k}")
                    G = gpool.tile([O, 2, 4, ckmax], f16, tag=f"g{k}")

                    xr = xs[:, offk : offk + ck]
                    rhs_t = sc[:, bp, 1, r0 : r0 + dk, :]
                    rhs_l = sc[:, bp, 1, r0 + 1 : r0 + 1 + dk, :]

                    plane_of = {GG: 0, GI: 1, GF: 2, GO: 3}
                    for g in (GG, GI, GF, GO):
                        p = plane_of[g]
                        lx = wxT_s[:, g * O : (g + 1) * O]
                        lw = whT_s[:, g * O : (g + 1) * O]
                        nc.tensor.matmul(
                            P[:, 0, p, :ck], lx, xr, start=True, stop=False
                        )
                        nc.tensor.matmul(
                            P[:, 1, p, :ck], lx, xr, start=True, stop=False
                        )
                        nc.tensor.matmul(
                            P[:, 0, p, :ck], lw, rhs_t, start=False, stop=True
                        )
                        nc.tensor.matmul(
                            P[:, 1, p, :ck], lw, rhs_l, start=False, stop=True
                        )

                    # planes are [g,i,f,o]; in latency-bound ramp wavefronts
                    # split out the o-gate sigmoid (only needed at pp) so it
                    # overlaps the DVE cn work; elsewhere one merged sigmoid
                    if nck == 1:
                        nc.scalar.activation(G[:, :, 0:3, :ck], P[:, :, 0:3, :ck], AF.Sigmoid)
                        nc.scalar.activation(G[:, :, 3, :ck], P[:, :, 3, :ck], AF.Sigmoid)
                    else:
                        nc.scalar.activation(G[:, :, 0:4, :ck], P[:, :, 0:4, :ck], AF.Sigmoid)

                    T1 = wpool.tile([O, 2, ckmax], f16, tag=f"t1{k}")
                    T2 = wpool.tile([O, 2, ckmax], f16, tag=f"t2{k}")
                    E = wpool.tile([O, 2, ckmax], f16, tag=f"e{k}")
                    o1 = offk - off

                    # g = 2*sigmoid(2x) - 1 affine fix, in place
                    nc.vector.tensor_scalar(
                        G[:, :, 0, :ck], G[:, :, 0, :ck], 2.0, -1.0, ALU.mult, ALU.add
                    )
                    # t2 = i*g (both branches in one op)
                    nc.vector.tensor_tensor(
                        T2[:, :, :ck], G[:, :, 1, :ck], G[:, :, 0, :ck], ALU.mult
                    )
                    # t1 = f*c_pred
                    nc.vector.tensor_tensor(
                        T1[:, 0, :ck], G[:, 0, 2, :ck], sc[:, bp, 0, r0 : r0 + dk, :], ALU.mult
                    )
                    nc.vector.tensor_tensor(
                        T1[:, 1, :ck], G[:, 1, 2, :ck], sc[:, bp, 0, r0 + 1 : r0 + 1 + dk, :], ALU.mult
                    )
                    # cn = t1 + t2 -> CP[0:2]
                    nc.vector.tensor_tensor(
                        CP[:, 0:2, o1 : o1 + ck], T1[:, :, :ck], T2[:, :, :ck], ALU.add
                    )
                    # tau = tanh(cn)
                    nc.scalar.activation(
                        TAU[:, :, o1 : o1 + ck], CP[:, 0:2, o1 : o1 + ck], AF.Tanh
                    )
                    # pp = o*tau -> CP[2:4]
                    nc.vector.tensor_tensor(
                        CP[:, 2:4, o1 : o1 + ck], G[:, :, 3, :ck], TAU[:, :, o1 : o1 + ck], ALU.mult
                    )
                    # E = ws0*[cn_t, pp_t] + bias, then
                    # new state [ct | ht] = ws1*[cn_l, pp_l] + E in one op
                    nc.vector.tensor_scalar(
                        E[:, :, :ck], CP[:, 0:4:2, o1 : o1 + ck], ws0_s[:], bias_s[:], ALU.mult, ALU.add
                    )
                    nc.vector.scalar_tensor_tensor(
                        sc[:, bc, 0:2, r0 + 1 : r0 + 1 + dk, :],
                        CP[:, 1:4:2, o1 : o1 + ck],
                        ws1_s[:],
                        E[:, :, :ck],
                        ALU.mult,
                        ALU.add,
                    )

                # stream this wavefront's h out in one DMA
                nc.sync.dma_start(
                    h_diag[:, off : off + db],
                    sc[:, bc, 1, i0 + 1 : i1 + 1, :],
                )

    nc.compile()
    return nc


# ---------------------------------------------------------------- host side


def _diag_index(h, w):
    cells = []
    for t, i0, i1, _ in _wavefronts(h, w):
        for i in range(i0, i1):
            cells.append((i, t - i))
    return np.array(cells)


def _prep_core_inputs(inputs, d, half, h, w):
    flips = [(False, False), (False, True), (True, False), (True, True)]
    fy, fx = flips[d]
    xd = inputs["x"][half * B : (half + 1) * B]  # (B, CIN, H, W)
    if fy:
        xd = xd[:, :, ::-1, :]
    if fx:
        xd = xd[:, :, :, ::-1]
    x_hw = np.ascontiguousarray(np.transpose(xd, (1, 2, 3, 0)))  # (CIN, H, W, B)

    cells = _diag_index(h, w)
    x_cells = x_hw[:, cells[:, 0], cells[:, 1], :].reshape(CIN, h * w * B)
    x_diag = np.ones((CIN + 1, h * w * B), np.float16)
    x_diag[:CIN] = x_cells.astype(np.float16)

    # gate order [i, f, o, g]
    gw_h = [inputs["w_hi"][d], inputs["w_hf"][d], inputs["w_ho"][d], inputs["w_hg"][d]]
    gw_x = [inputs["w_ii"][d], inputs["w_if"][d], inputs["w_io"][d], inputs["w_ig"][d]]
    gb = [inputs["b_i"][d], inputs["b_f"][d], inputs["b_o"][d], inputs["b_g"][d]]

    # gate g (block 3) weights doubled: kernel computes tanh via 2*sig(2x)-1
    whT = np.concatenate(
        [wh.T * (2.0 if g == 3 else 1.0) for g, wh in enumerate(gw_h)], axis=1
    ).astype(np.float16)
    wxT = np.zeros((CIN + 1, NG * O), np.float16)
    for g in range(NG):
        s = 2.0 if g == 3 else 1.0
        wxT[:CIN, g * O : (g + 1) * O] = (gw_x[g].T * s).astype(np.float16)
        wxT[CIN, g * O : (g + 1) * O] = (gb[g] * s).astype(np.float16)

    ws = inputs["weighted_sum"][d]
    return {
        "x_diag": x_diag,
        "whT": whT,
        "wxT": wxT,
        "ws0v": np.full((O, 1), ws[0], np.float32),
        "ws1v": np.full((O, 1), ws[1], np.float32),
        "biasv": np.asarray(inputs["bias"][d], np.float32).reshape(O, 1),
        "zerov": np.zeros((O, 2, 2, h + 1, B), np.float16),
    }


def _assemble_output(results, h, w):
    flips = [(False, False), (False, True), (True, False), (True, True)]
    cells = _diag_index(h, w)
    inv = np.empty(h * w, np.int64)
    inv[cells[:, 0] * w + cells[:, 1]] = np.arange(h * w)

    out = np.empty((NG, O, B_FULL, h, w), np.float32)
    for d in range(4):
        fy, fx = flips[d]
        for half in range(2):
            hd = results[d * 2 + half]["h_diag"].astype(np.float32)
            hv = hd.reshape(O, h * w, B)[:, inv, :].reshape(O, h, w, B)
            if fy:
                hv = hv[:, ::-1, :, :]
            if fx:
                hv = hv[:, :, ::-1, :]
            out[d, :, half * B : (half + 1) * B] = np.transpose(hv, (0, 3, 1, 2))
    return out


_module_cache = {}


def _get_module(h=H, w=W, chunks=CHUNKS):
    key = (h, w, chunks)
    if key not in _module_cache:
        _module_cache[key] = build_module(h, w, chunks)
    return _module_cache[key]


def make_in_maps(inputs, h=H, w=W):
    return [
        _prep_core_inputs(inputs, core // 2, core % 2, h, w) for core in range(N_CORES)
    ]


def kernel(**inputs) -> np.ndarray:
    from concourse import bass_utils

    nc = _get_module(H, W)
    in_maps = make_in_maps(inputs)
    res = bass_utils.run_bass_kernel_spmd(nc, in_maps, core_ids=list(range(N_CORES)))
    return _assemble_output(res.results, H, W)

